# revision 1
# baseline (speedup 1.0000x reference)
"""Distributed Trainium2 kernel for nn_Attention_72722386256499.

Full inputs in, full output out.  Internally shards the 32 (B,H)
attention problems over 8 NeuronCores: core m handles batch m//2,
heads [4*(m%2), 4*(m%2)+4).  The small 1x1-conv weights are sliced and
replicated host-side; the output projection is computed as per-core
partial products summed on the host (data-parallel reduce in unshard).
"""

import sys

sys.path.insert(0, "/opt/trn_rl_repo")

import numpy as np

import bass_rust
import concourse.bass as bass
import concourse.mybir as mybir
import concourse.tile as tile
from concourse import masks
from concourse.bass_utils import run_bass_kernel_spmd

B, C, L = 4, 512, 2048
H, D = 8, 64
HPC = 4  # heads per core
NCORES = 8
FP = mybir.dt.float32

# Matmul compute dtype: float32 (4 cyc/row) or float32r (1 cyc/row @ N>=256).
MM_DT = mybir.dt.float32

TRACE_MODE = False
LAST_RESULT = None
_NC_CACHE = {}


def _split_waits(nc, max_waits=1):
    """walrus here rejects >1 sync wait per instruction; hoist extras onto
    single-wait NoOps just before the instruction on the same engine."""
    counter = 0
    for f in nc.m.functions:
        for bb in f.blocks:
            il = bb.instructions
            new_list = []
            changed = False
            for inst in il:
                si = inst.sync_info
                if si is None:
                    new_list.append(inst)
                    continue
                waits = list(si.on_wait)
                if len(waits) > max_waits:
                    keep = waits[-max_waits:]
                    for w in waits[:-max_waits]:
                        counter += 1
                        nop = mybir.InstNoOp(
                            name=f"I-waitsplit-{counter}", ins=[], outs=[]
                        )
                        nop.engine = inst.engine
                        nop.sync_info = bass_rust.SyncInfo(on_wait=[w], on_update=[])
                        new_list.append(nop)
                        nc.register_instruction(nop, overwrite=True)
                    inst.sync_info = bass_rust.SyncInfo(
                        on_wait=keep, on_update=list(si.on_update)
                    )
                    changed = True
                new_list.append(inst)
            if changed:
                il.clear()
                il.extend(new_list)
    return counter


def _mm(nc, out, lhsT, rhs, start, stop):
    if MM_DT != mybir.dt.float32:
        lhsT = lhsT.bitcast(MM_DT)
        rhs = rhs.bitcast(MM_DT)
    nc.tensor.matmul(out, lhsT, rhs, start=start, stop=stop)


def build_nc():
    nc = bass.Bass()
    x_ext = nc.declare_dram_parameter("x", [C, L], FP, isOutput=False)
    wq_ext = nc.declare_dram_parameter("wq", [C, HPC * D], FP, isOutput=False)
    wk_ext = nc.declare_dram_parameter("wk", [C, HPC * D], FP, isOutput=False)
    wv_ext = nc.declare_dram_parameter("wv", [C, HPC * D], FP, isOutput=False)
    wo_ext = nc.declare_dram_parameter("wo", [HPC * D, C], FP, isOutput=False)
    out_ext = nc.declare_dram_parameter("out", [C, L], FP, isOutput=True)
    scratch = nc.dram_tensor("scratch", [HPC, L, D], FP)

    NJ = L // 128  # 16 j tiles per head
    NIB = L // 512  # 4 i blocks per head
    NCC = C // 128  # 4 contraction chunks

    with tile.TileContext(nc) as tc:
        with (
            tc.tile_pool(name="const", bufs=1) as cpool,
            tc.tile_pool(name="exp", bufs=3) as epool,
            tc.tile_pool(name="o2", bufs=2) as o2pool,
            tc.tile_pool(name="rz", bufs=4) as rzpool,
            tc.tile_pool(name="fout", bufs=3) as fpool,
            tc.tile_pool(name="ps", bufs=2, space="PSUM") as ppool,
        ):
            # ---- persistent SBUF tensors ----
            x_sb = cpool.tile([128, NCC, L], FP, tag="x")
            wq_sb = cpool.tile([128, NCC, HPC * D], FP, tag="wq")
            wk_sb = cpool.tile([128, NCC, HPC * D], FP, tag="wk")
            wv_sb = cpool.tile([128, NCC, HPC * D], FP, tag="wv")
            wo_sb = cpool.tile([128, 2, C], FP, tag="wo")
            q_sb = cpool.tile([128, 2, L], FP, tag="q")
            k_sb = cpool.tile([128, 2, L], FP, tag="k")
            vT1_sb = cpool.tile([128, NJ, HPC, D + 1], FP, tag="vT1")
            out2_sb = cpool.tile([128, HPC, NJ, D], FP, tag="out2")
            outrs_sb = cpool.tile([128, 2, L], FP, tag="outrs")
            ident = cpool.tile([128, 128], FP, tag="ident")

            masks.make_identity(nc, ident[:, :])
            nc.vector.memset(vT1_sb[:, :, :, D : D + 1], 1.0)

            # ---- phase A: DMA inputs, project q/k/vT ----
            nc.sync.dma_start(
                out=x_sb, in_=x_ext.rearrange("(ci p) l -> p ci l", p=128)
            )
            nc.sync.dma_start(
                out=wq_sb, in_=wq_ext.rearrange("(ci p) n -> p ci n", p=128)
            )
            nc.sync.dma_start(
                out=wk_sb, in_=wk_ext.rearrange("(ci p) n -> p ci n", p=128)
            )
            nc.sync.dma_start(
                out=wv_sb, in_=wv_ext.rearrange("(ci p) n -> p ci n", p=128)
            )
            nc.sync.dma_start(
                out=wo_sb, in_=wo_ext.rearrange("(rc p) o -> p rc o", p=128)
            )

            # q, k: (128 rows = head pair) x L, per group g
            for w_sb, dst in ((wq_sb, q_sb), (wk_sb, k_sb)):
                for g in range(2):
                    for lb in range(NIB):
                        ps = ppool.tile([128, 512], FP, tag="s")
                        for ci in range(NCC):
                            _mm(
                                nc,
                                ps,
                                w_sb[:, ci, g * 128 : (g + 1) * 128],
                                x_sb[:, ci, lb * 512 : (lb + 1) * 512],
                                start=(ci == 0),
                                stop=(ci == NCC - 1),
                            )
                        nc.vector.tensor_copy(
                            out=dst[:, g, lb * 512 : (lb + 1) * 512], in_=ps
                        )

            # vT: (128 l) x (4 heads * 64 d), per j tile
            for jt in range(NJ):
                ps = ppool.tile([128, HPC * D], FP, tag="s")
                for ci in range(NCC):
                    _mm(
                        nc,
                        ps,
                        x_sb[:, ci, jt * 128 : (jt + 1) * 128],
                        wv_sb[:, ci, :],
                        start=(ci == 0),
                        stop=(ci == NCC - 1),
                    )
                nc.vector.tensor_copy(
                    out=vT1_sb[:, jt, :, 0:D],
                    in_=ps.rearrange("p (h d) -> p h d", h=HPC),
                )

            # ---- phase B/C: attention per (head, i-block) ----
            for h in range(HPC):
                g, hp = h // 2, h % 2
                p0 = hp * 64
                for ib in range(NIB):
                    i0 = ib * 512
                    ps_o = ppool.tile([128, 512], FP, tag="o")
                    for jj in range(NJ // 2):
                        ps_s = ppool.tile([128, 1024], FP, tag="s")
                        for t2 in range(2):
                            jt = jj * 2 + t2
                            _mm(
                                nc,
                                ps_s[:, t2 * 512 : (t2 + 1) * 512],
                                k_sb[p0 : p0 + 64, g, jt * 128 : (jt + 1) * 128],
                                q_sb[p0 : p0 + 64, g, i0 : i0 + 512],
                                start=True,
                                stop=True,
                            )
                        ex = epool.tile([128, 1024], FP, tag="exp")
                        nc.scalar.activation(
                            out=ex, in_=ps_s, func=mybir.ActivationFunctionType.Exp
                        )
                        for t2 in range(2):
                            jt = jj * 2 + t2
                            _mm(
                                nc,
                                ps_o[0 : D + 1, :],
                                vT1_sb[:, jt, h, :],
                                ex[:, t2 * 512 : (t2 + 1) * 512],
                                start=(jj == 0 and t2 == 0),
                                stop=(jj == NJ // 2 - 1 and t2 == 1),
                            )
                    o2T = o2pool.tile([128, 512], FP, tag="o2T")
                    nc.vector.tensor_copy(
                        out=o2T[0 : D + 1, :], in_=ps_o[0 : D + 1, :]
                    )
                    # transpose back to (i, d) + normalize by the ones-row sum
                    for cc in range(4):
                        tg = ib * 4 + cc
                        ps_t = ppool.tile([128, 512], FP, tag="t")
                        nc.tensor.transpose(
                            ps_t[:, 0:128],
                            o2T[:, cc * 128 : (cc + 1) * 128],
                            ident[:, :],
                        )
                        rz = rzpool.tile([128, 1], FP, tag="rz")
                        nc.vector.reciprocal(out=rz, in_=ps_t[:, D : D + 1])
                        nc.vector.tensor_scalar_mul(
                            out=out2_sb[:, h, tg, :],
                            in0=ps_t[:, 0:D],
                            scalar1=rz,
                        )
                    nc.sync.dma_start(
                        out=scratch[h, i0 : i0 + 512, :].rearrange(
                            "(cc p) d -> p cc d", p=128
                        ),
                        in_=out2_sb[:, h, ib * 4 : (ib + 1) * 4, :],
                    )

            # ---- phase D: funky-reshape view + output projection ----
            scratch_rs = scratch.ap().flatten().rearrange("(q e) -> q e", e=L)
            for g2 in range(2):
                nc.sync.dma_start(
                    out=outrs_sb[:, g2, :],
                    in_=scratch_rs[g2 * 128 : (g2 + 1) * 128, :],
                )
            for og in range(4):
                for lb in range(NIB):
                    ps_f = ppool.tile([128, 512], FP, tag="o")
                    for rc in range(2):
                        _mm(
                            nc,
                            ps_f,
                            wo_sb[:, rc, og * 128 : (og + 1) * 128],
                            outrs_sb[:, rc, lb * 512 : (lb + 1) * 512],
                            start=(rc == 0),
                            stop=(rc == 1),
                        )
                    fo = fpool.tile([128, 512], FP, tag="fout")
                    nc.scalar.copy(out=fo, in_=ps_f)
                    nc.sync.dma_start(
                        out=out_ext[
                            og * 128 : (og + 1) * 128, lb * 512 : (lb + 1) * 512
                        ],
                        in_=fo,
                    )

    _split_waits(nc)
    return nc


def _get_nc():
    key = str(MM_DT)
    if key not in _NC_CACHE:
        _NC_CACHE[key] = build_nc()
    return _NC_CACHE[key]


def kernel(x, w_qkv, w_out, b_out):
    global LAST_RESULT
    x = np.asarray(x, dtype=np.float32)
    w_qkv = np.asarray(w_qkv, dtype=np.float32)
    w_out = np.asarray(w_out, dtype=np.float32)
    b_out = np.asarray(b_out, dtype=np.float32)

    scale = D**-0.5
    in_maps = []
    for m in range(NCORES):
        b = m // 2
        hs = [4 * (m % 2) + i for i in range(HPC)]
        q_rows = np.concatenate([np.arange(h * D, (h + 1) * D) for h in hs])
        wq = np.ascontiguousarray((w_qkv[q_rows, :] * scale).T)
        wk = np.ascontiguousarray(w_qkv[C + q_rows, :].T)
        wv = np.ascontiguousarray(w_qkv[2 * C + q_rows, :].T)
        wo = np.ascontiguousarray(w_out[:, q_rows].T)
        in_maps.append(
            {
                "x": np.ascontiguousarray(x[b]),
                "wq": wq,
                "wk": wk,
                "wv": wv,
                "wo": wo,
            }
        )

    nc = _get_nc()
    res = run_bass_kernel_spmd(
        nc, in_maps, core_ids=list(range(NCORES)), trace=TRACE_MODE
    )
    LAST_RESULT = res

    out = np.empty((B, C, L), dtype=np.float32)
    for b in range(B):
        out[b] = res.results[2 * b]["out"] + res.results[2 * b + 1]["out"]
        out[b] += b_out[:, None]
    return out


# revision 8
# speedup vs baseline: 2.8818x; 2.8818x over previous
"""Distributed Trainium2 kernel for nn_Attention_72722386256499.

Full inputs in, full output out.  Internally shards the 32 (B,H)
attention problems over 8 NeuronCores: core m handles batch m//2,
heads [4*(m%2), 4*(m%2)+4).  The small 1x1-conv weights are sliced and
replicated host-side; the output projection is computed as per-core
partial products summed on the host (data-parallel reduce in unshard).
"""

import sys

sys.path.insert(0, "/opt/trn_rl_repo")

import numpy as np

import bass_rust
import concourse.bass as bass
import concourse.mybir as mybir
import concourse.tile as tile
from concourse import masks
from concourse.bass_utils import run_bass_kernel_spmd

B, C, L = 4, 512, 2048
H, D = 8, 64
HPC = 4  # heads per core
NCORES = 8
FP = mybir.dt.float32

# Matmul compute dtype: float32 (4 cyc/row) or float32r (1 cyc/row @ N>=256).
MM_DT = mybir.dt.float32r

TRACE_MODE = False
LAST_RESULT = None
_NC_CACHE = {}


def _split_waits(nc, max_waits=1):
    """walrus here rejects >1 sync wait per instruction; hoist extras onto
    single-wait NoOps just before the instruction on the same engine."""
    counter = 0
    for f in nc.m.functions:
        for bb in f.blocks:
            il = bb.instructions
            new_list = []
            changed = False
            for inst in il:
                si = inst.sync_info
                if si is None:
                    new_list.append(inst)
                    continue
                waits = list(si.on_wait)
                if len(waits) > max_waits:
                    keep = waits[-max_waits:]
                    for w in waits[:-max_waits]:
                        counter += 1
                        nop = mybir.InstNoOp(
                            name=f"I-waitsplit-{counter}", ins=[], outs=[]
                        )
                        nop.engine = inst.engine
                        nop.sync_info = bass_rust.SyncInfo(on_wait=[w], on_update=[])
                        new_list.append(nop)
                        nc.register_instruction(nop, overwrite=True)
                    inst.sync_info = bass_rust.SyncInfo(
                        on_wait=keep, on_update=list(si.on_update)
                    )
                    changed = True
                new_list.append(inst)
            if changed:
                il.clear()
                il.extend(new_list)
    return counter


def _mm(nc, out, lhsT, rhs, start, stop):
    nc.tensor.matmul(out, lhsT, rhs, start=start, stop=stop)


def build_nc():
    nc = bass.Bass()
    x_ext = nc.declare_dram_parameter("x", [C, L], FP, isOutput=False)
    wq_ext = nc.declare_dram_parameter("wq", [C, HPC * D], FP, isOutput=False)
    wk_ext = nc.declare_dram_parameter("wk", [C, HPC * D], FP, isOutput=False)
    wv_ext = nc.declare_dram_parameter("wv", [C, HPC * D], FP, isOutput=False)
    wo_ext = nc.declare_dram_parameter("wo", [HPC * D, C], FP, isOutput=False)
    out_ext = nc.declare_dram_parameter("out", [C, L], FP, isOutput=True)
    scratch = nc.dram_tensor("scratch", [HPC, L, D], FP)

    NJ = L // 128  # 16 j tiles per head
    NIB = L // 512  # 4 i blocks per head
    NCC = C // 128  # 4 contraction chunks

    with tile.TileContext(nc) as tc:
        with (
            tc.tile_pool(name="const", bufs=1) as cpool,
            tc.tile_pool(name="exp", bufs=3) as epool,
            tc.tile_pool(name="o2", bufs=2) as o2pool,
            tc.tile_pool(name="rz", bufs=4) as rzpool,
            tc.tile_pool(name="fout", bufs=3) as fpool,
            tc.tile_pool(name="ps", bufs=2, space="PSUM") as ppool,
        ):
            # ---- persistent SBUF tensors ----
            # MDT tensors feed TensorE matmuls; float32r must be rounded
            # at the producing instruction (DMA/copy/activation output).
            MDT = MM_DT
            x_sb = cpool.tile([128, NCC, L], MDT, tag="x")
            wq_sb = cpool.tile([128, NCC, HPC * D], MDT, tag="wq")
            wk_sb = cpool.tile([128, NCC, HPC * D], MDT, tag="wk")
            wv_sb = cpool.tile([128, NCC, HPC * D], MDT, tag="wv")
            wo_sb = cpool.tile([128, 2, C], MDT, tag="wo")
            q_sb = cpool.tile([128, 2, L], MDT, tag="q")
            k_sb = cpool.tile([128, 2, L], MDT, tag="k")
            vT1_sb = cpool.tile([128, NJ, HPC, D + 1], MDT, tag="vT1")
            out2_sb = cpool.tile([128, HPC, NJ, D], FP, tag="out2")
            outrs_sb = cpool.tile([128, 2, L], MDT, tag="outrs")
            ident = cpool.tile([128, 128], FP, tag="ident")

            masks.make_identity(nc, ident[:, :])
            ones_f32 = cpool.tile([128, NJ * HPC], FP, tag="ones")
            nc.vector.memset(ones_f32, 1.0)
            nc.vector.tensor_copy(
                out=vT1_sb[:, :, :, D : D + 1],
                in_=ones_f32.rearrange("p (a b) -> p a b", b=HPC).unsqueeze(-1),
            )

            # ---- phase A: DMA inputs, project q/k/vT ----
            nc.gpsimd.dma_start(
                out=x_sb, in_=x_ext.rearrange("(ci p) l -> p ci l", p=128)
            )
            nc.gpsimd.dma_start(
                out=wq_sb, in_=wq_ext.rearrange("(ci p) n -> p ci n", p=128)
            )
            nc.gpsimd.dma_start(
                out=wk_sb, in_=wk_ext.rearrange("(ci p) n -> p ci n", p=128)
            )
            nc.gpsimd.dma_start(
                out=wv_sb, in_=wv_ext.rearrange("(ci p) n -> p ci n", p=128)
            )
            nc.gpsimd.dma_start(
                out=wo_sb, in_=wo_ext.rearrange("(rc p) o -> p rc o", p=128)
            )

            # q, k: (128 rows = head pair) x L, per group g
            for w_sb, dst in ((wq_sb, q_sb), (wk_sb, k_sb)):
                for g in range(2):
                    for lb in range(NIB):
                        ps = ppool.tile([128, 512], FP, tag="s")
                        for ci in range(NCC):
                            _mm(
                                nc,
                                ps,
                                w_sb[:, ci, g * 128 : (g + 1) * 128],
                                x_sb[:, ci, lb * 512 : (lb + 1) * 512],
                                start=(ci == 0),
                                stop=(ci == NCC - 1),
                            )
                        nc.vector.tensor_copy(
                            out=dst[:, g, lb * 512 : (lb + 1) * 512], in_=ps
                        )

            # vT: (128 l) x (4 heads * 64 d), per j tile
            for jt in range(NJ):
                ps = ppool.tile([128, HPC * D], FP, tag="s")
                for ci in range(NCC):
                    _mm(
                        nc,
                        ps,
                        x_sb[:, ci, jt * 128 : (jt + 1) * 128],
                        wv_sb[:, ci, :],
                        start=(ci == 0),
                        stop=(ci == NCC - 1),
                    )
                nc.vector.tensor_copy(
                    out=vT1_sb[:, jt, :, 0:D],
                    in_=ps.rearrange("p (h d) -> p h d", h=HPC),
                )

            # ---- phase B/C: attention per (head, i-block) ----
            for h in range(HPC):
                g, hp = h // 2, h % 2
                p0 = hp * 64
                for ib in range(NIB):
                    i0 = ib * 512
                    ps_o = ppool.tile([128, 512], FP, tag="o")
                    for jj in range(NJ // 2):
                        ps_s = ppool.tile([128, 1024], FP, tag="s")
                        for t2 in range(2):
                            jt = jj * 2 + t2
                            _mm(
                                nc,
                                ps_s[:, t2 * 512 : (t2 + 1) * 512],
                                k_sb[p0 : p0 + 64, g, jt * 128 : (jt + 1) * 128],
                                q_sb[p0 : p0 + 64, g, i0 : i0 + 512],
                                start=True,
                                stop=True,
                            )
                        ex = epool.tile([128, 1024], MDT, tag="exp")
                        nc.scalar.activation(
                            out=ex, in_=ps_s, func=mybir.ActivationFunctionType.Exp
                        )
                        for t2 in range(2):
                            jt = jj * 2 + t2
                            _mm(
                                nc,
                                ps_o[0 : D + 1, :],
                                vT1_sb[:, jt, h, :],
                                ex[:, t2 * 512 : (t2 + 1) * 512],
                                start=(jj == 0 and t2 == 0),
                                stop=(jj == NJ // 2 - 1 and t2 == 1),
                            )
                    o2T = o2pool.tile([128, 512], FP, tag="o2T")
                    nc.vector.tensor_copy(
                        out=o2T[0 : D + 1, :], in_=ps_o[0 : D + 1, :]
                    )
                    # transpose back to (i, d) + normalize by the ones-row sum
                    for cc in range(4):
                        tg = ib * 4 + cc
                        ps_t = ppool.tile([128, 512], FP, tag="t")
                        nc.tensor.transpose(
                            ps_t[:, 0:128],
                            o2T[:, cc * 128 : (cc + 1) * 128],
                            ident[:, :],
                        )
                        rz = rzpool.tile([128, 1], FP, tag="rz")
                        nc.vector.reciprocal(out=rz, in_=ps_t[:, D : D + 1])
                        nc.vector.tensor_scalar_mul(
                            out=out2_sb[:, h, tg, :],
                            in0=ps_t[:, 0:D],
                            scalar1=rz,
                        )
                    nc.sync.dma_start(
                        out=scratch[h, i0 : i0 + 512, :].rearrange(
                            "(cc p) d -> p cc d", p=128
                        ),
                        in_=out2_sb[:, h, ib * 4 : (ib + 1) * 4, :],
                    )

            # ---- phase D: funky-reshape view + output projection ----
            scratch_rs = scratch.ap().flatten().rearrange("(q e) -> q e", e=L)
            for g2 in range(2):
                nc.gpsimd.dma_start(
                    out=outrs_sb[:, g2, :],
                    in_=scratch_rs[g2 * 128 : (g2 + 1) * 128, :],
                )
            for og in range(4):
                for lb in range(NIB):
                    ps_f = ppool.tile([128, 512], FP, tag="o")
                    for rc in range(2):
                        _mm(
                            nc,
                            ps_f,
                            wo_sb[:, rc, og * 128 : (og + 1) * 128],
                            outrs_sb[:, rc, lb * 512 : (lb + 1) * 512],
                            start=(rc == 0),
                            stop=(rc == 1),
                        )
                    fo = fpool.tile([128, 512], FP, tag="fout")
                    nc.scalar.copy(out=fo, in_=ps_f)
                    nc.sync.dma_start(
                        out=out_ext[
                            og * 128 : (og + 1) * 128, lb * 512 : (lb + 1) * 512
                        ],
                        in_=fo,
                    )

    _split_waits(nc)
    return nc


def _get_nc():
    key = str(MM_DT)
    if key not in _NC_CACHE:
        _NC_CACHE[key] = build_nc()
    return _NC_CACHE[key]


def kernel(x, w_qkv, w_out, b_out):
    global LAST_RESULT
    x = np.asarray(x, dtype=np.float32)
    w_qkv = np.asarray(w_qkv, dtype=np.float32)
    w_out = np.asarray(w_out, dtype=np.float32)
    b_out = np.asarray(b_out, dtype=np.float32)

    scale = D**-0.5
    in_maps = []
    for m in range(NCORES):
        b = m // 2
        hs = [4 * (m % 2) + i for i in range(HPC)]
        q_rows = np.concatenate([np.arange(h * D, (h + 1) * D) for h in hs])
        wq = np.ascontiguousarray((w_qkv[q_rows, :] * scale).T)
        wk = np.ascontiguousarray(w_qkv[C + q_rows, :].T)
        wv = np.ascontiguousarray(w_qkv[2 * C + q_rows, :].T)
        wo = np.ascontiguousarray(w_out[:, q_rows].T)
        in_maps.append(
            {
                "x": np.ascontiguousarray(x[b]),
                "wq": wq,
                "wk": wk,
                "wv": wv,
                "wo": wo,
            }
        )

    nc = _get_nc()
    res = run_bass_kernel_spmd(
        nc, in_maps, core_ids=list(range(NCORES)), trace=TRACE_MODE
    )
    LAST_RESULT = res

    out = np.empty((B, C, L), dtype=np.float32)
    for b in range(B):
        out[b] = res.results[2 * b]["out"] + res.results[2 * b + 1]["out"]
        out[b] += b_out[:, None]
    return out


# revision 10
# speedup vs baseline: 3.0868x; 1.0712x over previous
"""Distributed Trainium2 kernel for nn_Attention_72722386256499.

Full inputs in, full output out.  Internally shards the 32 (B,H)
attention problems over 8 NeuronCores: core m handles batch m//2,
heads [4*(m%2), 4*(m%2)+4).  The small 1x1-conv weights are sliced and
replicated host-side; the output projection is computed as per-core
partial products summed on the host (data-parallel reduce in unshard).
"""

import sys

sys.path.insert(0, "/opt/trn_rl_repo")

import numpy as np

import bass_rust
import concourse.bass as bass
import concourse.mybir as mybir
import concourse.tile as tile
from concourse import masks
from concourse.bass_utils import run_bass_kernel_spmd

B, C, L = 4, 512, 2048
H, D = 8, 64
HPC = 4  # heads per core
NCORES = 8
FP = mybir.dt.float32

# Matmul compute dtype: float32 (4 cyc/row) or float32r (1 cyc/row @ N>=256).
MM_DT = mybir.dt.bfloat16

TRACE_MODE = False
LAST_RESULT = None
_NC_CACHE = {}


def _split_waits(nc, max_waits=1):
    """walrus here rejects >1 sync wait per instruction; hoist extras onto
    single-wait NoOps just before the instruction on the same engine."""
    counter = 0
    for f in nc.m.functions:
        for bb in f.blocks:
            il = bb.instructions
            new_list = []
            changed = False
            for inst in il:
                si = inst.sync_info
                if si is None:
                    new_list.append(inst)
                    continue
                waits = list(si.on_wait)
                if len(waits) > max_waits:
                    keep = waits[-max_waits:]
                    for w in waits[:-max_waits]:
                        counter += 1
                        nop = mybir.InstNoOp(
                            name=f"I-waitsplit-{counter}", ins=[], outs=[]
                        )
                        nop.engine = inst.engine
                        nop.sync_info = bass_rust.SyncInfo(on_wait=[w], on_update=[])
                        new_list.append(nop)
                        nc.register_instruction(nop, overwrite=True)
                    inst.sync_info = bass_rust.SyncInfo(
                        on_wait=keep, on_update=list(si.on_update)
                    )
                    changed = True
                new_list.append(inst)
            if changed:
                il.clear()
                il.extend(new_list)
    return counter


def _mm(nc, out, lhsT, rhs, start, stop):
    nc.tensor.matmul(out, lhsT, rhs, start=start, stop=stop)


def build_nc():
    nc = bass.Bass()
    x_ext = nc.declare_dram_parameter("x", [C, L], FP, isOutput=False)
    wq_ext = nc.declare_dram_parameter("wq", [C, HPC * D], FP, isOutput=False)
    wk_ext = nc.declare_dram_parameter("wk", [C, HPC * D], FP, isOutput=False)
    wv_ext = nc.declare_dram_parameter("wv", [C, HPC * D], FP, isOutput=False)
    wo_ext = nc.declare_dram_parameter("wo", [HPC * D, C], FP, isOutput=False)
    out_ext = nc.declare_dram_parameter("out", [C, L], FP, isOutput=True)
    scratch = nc.dram_tensor("scratch", [HPC, L, D], FP)

    NJ = L // 128  # 16 j tiles per head
    NIB = L // 512  # 4 i blocks per head
    NCC = C // 128  # 4 contraction chunks

    with tile.TileContext(nc) as tc:
        with (
            tc.tile_pool(name="const", bufs=1) as cpool,
            tc.tile_pool(name="exp", bufs=3) as epool,
            tc.tile_pool(name="o2", bufs=2) as o2pool,
            tc.tile_pool(name="rz", bufs=4) as rzpool,
            tc.tile_pool(name="fout", bufs=3) as fpool,
            tc.tile_pool(name="ps", bufs=2, space="PSUM") as ppool,
        ):
            # ---- persistent SBUF tensors ----
            # MDT tensors feed TensorE matmuls; float32r must be rounded
            # at the producing instruction (DMA/copy/activation output).
            MDT = MM_DT
            x_sb = cpool.tile([128, NCC, L], MDT, tag="x")
            wq_sb = cpool.tile([128, NCC, HPC * D], MDT, tag="wq")
            wk_sb = cpool.tile([128, NCC, HPC * D], MDT, tag="wk")
            wv_sb = cpool.tile([128, NCC, HPC * D], MDT, tag="wv")
            wo_sb = cpool.tile([128, 2, C], MDT, tag="wo")
            q_sb = cpool.tile([128, 2, L], MDT, tag="q")
            k_sb = cpool.tile([128, 2, L], MDT, tag="k")
            vT1_sb = cpool.tile([128, NJ, HPC, D + 1], MDT, tag="vT1")
            out2_sb = cpool.tile([128, HPC, NJ, D], FP, tag="out2")
            outrs_sb = cpool.tile([128, 2, L], MDT, tag="outrs")
            ident = cpool.tile([128, 128], FP, tag="ident")

            masks.make_identity(nc, ident[:, :])
            ones_f32 = cpool.tile([128, NJ * HPC], FP, tag="ones")
            nc.vector.memset(ones_f32, 1.0)
            nc.vector.tensor_copy(
                out=vT1_sb[:, :, :, D : D + 1],
                in_=ones_f32.rearrange("p (a b) -> p a b", b=HPC).unsqueeze(-1),
            )

            # ---- phase A: DMA inputs, project q/k/vT ----
            nc.gpsimd.dma_start(
                out=x_sb, in_=x_ext.rearrange("(ci p) l -> p ci l", p=128)
            )
            nc.gpsimd.dma_start(
                out=wq_sb, in_=wq_ext.rearrange("(ci p) n -> p ci n", p=128)
            )
            nc.gpsimd.dma_start(
                out=wk_sb, in_=wk_ext.rearrange("(ci p) n -> p ci n", p=128)
            )
            nc.gpsimd.dma_start(
                out=wv_sb, in_=wv_ext.rearrange("(ci p) n -> p ci n", p=128)
            )
            nc.gpsimd.dma_start(
                out=wo_sb, in_=wo_ext.rearrange("(rc p) o -> p rc o", p=128)
            )

            # q, k: (128 rows = head pair) x L, per group g
            for w_sb, dst in ((wq_sb, q_sb), (wk_sb, k_sb)):
                for g in range(2):
                    for lb in range(NIB):
                        ps = ppool.tile([128, 512], FP, tag="s")
                        for ci in range(NCC):
                            _mm(
                                nc,
                                ps,
                                w_sb[:, ci, g * 128 : (g + 1) * 128],
                                x_sb[:, ci, lb * 512 : (lb + 1) * 512],
                                start=(ci == 0),
                                stop=(ci == NCC - 1),
                            )
                        nc.vector.tensor_copy(
                            out=dst[:, g, lb * 512 : (lb + 1) * 512], in_=ps
                        )

            # vT: (128 l) x (4 heads * 64 d), per j tile
            for jt in range(NJ):
                ps = ppool.tile([128, HPC * D], FP, tag="s")
                for ci in range(NCC):
                    _mm(
                        nc,
                        ps,
                        x_sb[:, ci, jt * 128 : (jt + 1) * 128],
                        wv_sb[:, ci, :],
                        start=(ci == 0),
                        stop=(ci == NCC - 1),
                    )
                nc.vector.tensor_copy(
                    out=vT1_sb[:, jt, :, 0:D],
                    in_=ps.rearrange("p (h d) -> p h d", h=HPC),
                )

            # ---- phase B/C: attention per (head-pair, i-block) ----
            # The two heads of a pair sit on partition halves 0:64 / 64:128,
            # so their K=64 S^T matmuls row-tile into disjoint PE quadrants
            # and run concurrently.
            for g in range(2):
                for ib in range(NIB):
                    i0 = ib * 512
                    ps_oA = ppool.tile([128, 512], FP, tag="o")
                    ps_oB = ppool.tile([128, 512], FP, tag="o")
                    for jt in range(NJ):
                        ps_s = ppool.tile([128, 1024], FP, tag="s")
                        for hp in range(2):
                            p0 = hp * 64
                            _mm(
                                nc,
                                ps_s[:, hp * 512 : (hp + 1) * 512],
                                k_sb[p0 : p0 + 64, g, jt * 128 : (jt + 1) * 128],
                                q_sb[p0 : p0 + 64, g, i0 : i0 + 512],
                                start=True,
                                stop=True,
                            )
                        ex = epool.tile([128, 1024], MDT, tag="exp")
                        nc.scalar.activation(
                            out=ex, in_=ps_s, func=mybir.ActivationFunctionType.Exp
                        )
                        for hp, ps_o in ((0, ps_oA), (1, ps_oB)):
                            _mm(
                                nc,
                                ps_o[0 : D + 1, :],
                                vT1_sb[:, jt, 2 * g + hp, :],
                                ex[:, hp * 512 : (hp + 1) * 512],
                                start=(jt == 0),
                                stop=(jt == NJ - 1),
                            )
                    for hp, ps_o in ((0, ps_oA), (1, ps_oB)):
                        h = 2 * g + hp
                        o2T = o2pool.tile([128, 512], FP, tag="o2T")
                        nc.vector.tensor_copy(
                            out=o2T[0 : D + 1, :], in_=ps_o[0 : D + 1, :]
                        )
                        # transpose to (i, d) + normalize by the ones-row sum
                        for cc in range(4):
                            tg = ib * 4 + cc
                            ps_t = ppool.tile([128, 512], FP, tag="t")
                            nc.tensor.transpose(
                                ps_t[:, 0:128],
                                o2T[:, cc * 128 : (cc + 1) * 128],
                                ident[:, :],
                            )
                            rz = rzpool.tile([128, 1], FP, tag="rz")
                            nc.vector.reciprocal(out=rz, in_=ps_t[:, D : D + 1])
                            nc.vector.tensor_scalar_mul(
                                out=out2_sb[:, h, tg, :],
                                in0=ps_t[:, 0:D],
                                scalar1=rz,
                            )
                        nc.sync.dma_start(
                            out=scratch[h, i0 : i0 + 512, :].rearrange(
                                "(cc p) d -> p cc d", p=128
                            ),
                            in_=out2_sb[:, h, ib * 4 : (ib + 1) * 4, :],
                        )

            # ---- phase D: funky-reshape view + output projection ----
            scratch_rs = scratch.ap().flatten().rearrange("(q e) -> q e", e=L)
            for g2 in range(2):
                nc.gpsimd.dma_start(
                    out=outrs_sb[:, g2, :],
                    in_=scratch_rs[g2 * 128 : (g2 + 1) * 128, :],
                )
            for og in range(4):
                for lb in range(NIB):
                    ps_f = ppool.tile([128, 512], FP, tag="o")
                    for rc in range(2):
                        _mm(
                            nc,
                            ps_f,
                            wo_sb[:, rc, og * 128 : (og + 1) * 128],
                            outrs_sb[:, rc, lb * 512 : (lb + 1) * 512],
                            start=(rc == 0),
                            stop=(rc == 1),
                        )
                    fo = fpool.tile([128, 512], FP, tag="fout")
                    nc.vector.tensor_copy(out=fo, in_=ps_f)
                    nc.sync.dma_start(
                        out=out_ext[
                            og * 128 : (og + 1) * 128, lb * 512 : (lb + 1) * 512
                        ],
                        in_=fo,
                    )

    _split_waits(nc)
    return nc


def _get_nc():
    key = str(MM_DT)
    if key not in _NC_CACHE:
        _NC_CACHE[key] = build_nc()
    return _NC_CACHE[key]


def kernel(x, w_qkv, w_out, b_out):
    global LAST_RESULT
    x = np.asarray(x, dtype=np.float32)
    w_qkv = np.asarray(w_qkv, dtype=np.float32)
    w_out = np.asarray(w_out, dtype=np.float32)
    b_out = np.asarray(b_out, dtype=np.float32)

    scale = D**-0.5
    in_maps = []
    for m in range(NCORES):
        b = m // 2
        hs = [4 * (m % 2) + i for i in range(HPC)]
        q_rows = np.concatenate([np.arange(h * D, (h + 1) * D) for h in hs])
        wq = np.ascontiguousarray((w_qkv[q_rows, :] * scale).T)
        wk = np.ascontiguousarray(w_qkv[C + q_rows, :].T)
        wv = np.ascontiguousarray(w_qkv[2 * C + q_rows, :].T)
        wo = np.ascontiguousarray(w_out[:, q_rows].T)
        in_maps.append(
            {
                "x": np.ascontiguousarray(x[b]),
                "wq": wq,
                "wk": wk,
                "wv": wv,
                "wo": wo,
            }
        )

    nc = _get_nc()
    res = run_bass_kernel_spmd(
        nc, in_maps, core_ids=list(range(NCORES)), trace=TRACE_MODE
    )
    LAST_RESULT = res

    out = np.empty((B, C, L), dtype=np.float32)
    for b in range(B):
        out[b] = res.results[2 * b]["out"] + res.results[2 * b + 1]["out"]
        out[b] += b_out[:, None]
    return out
